# revision 11
# baseline (speedup 1.0000x reference)
"""Trainium2 Bass kernel for batched weighted scatter-add (AttentionCopy).

Computes out[b, o, v] = sum_i attn[b, o, i] * (ids[b, i] == v)
for ids [16, 512] int32 in [0, 50000), attn [16, 32, 512] f32,
out [16, 32, 50000] f32.

Strategy: pure data parallel over the batch dim — 2 batches per core on 8
cores. Per batch the [32, 50000] output is built densely in 10 PSUM tiles of
[128, 1250], one per contiguous vocab span of 5000 = 4 groups x 1250. Tile
rows are o-major (o, gl) pairs (o in 0..31, gl in 0..3 local group), so each
tile's DRAM write is a [32, 4, 1250] access pattern whose outer dim (32)
spreads across all 16 SDMA engines (outer-dim count < 16 would leave engines
idle — measured 4x DMA slowdown with a g-major [4, 32, 1250] pattern).

The host buckets each batch's 512 ids into the 10 spans (index-only
preprocessing; uniform ids put ~51 of 512 in each span, max 67 observed,
capacity 128) and packs, per span, the stationary matrix

  gt[i, (o, gl)] = attn[b, o, orig_i] if hi_i == gl else 0     (fp16)

(a masked gather of attn columns — placement only, done on host) plus the
lo = span-relative id mod 1250 values. The device then does one K=128
matmul pass per tile against an on-device one-hot:

  out[(o, gl), lo] = gt.T @ alo,     alo[i, lo] = (lo_i == lo)

This cuts tensor-engine time ~4x vs the K=512 dense formulation (the
original bottleneck: 53us busy of 65us total) and leaves the kernel bounded
by the mandatory 12.8 MB/core f32 output write (~33us at the measured
~390 GB/s aggregate of the 16 SDMA engines, 22.5 GB/s each).

The remaining engine work is balanced so neither scalar nor vector exceeds
the DMA cadence: alo one-hots (vector, LA tiles ahead of the matmuls), and
the PSUM->SBUF copy of each tile column-split scalar [0:800] (slower ACT
copy, no other work) / vector [800:1250], each half DMA-kicked on its own
HWDGE queue. The lo-compare iota constant is generated by gpsimd at t=0.
"""

import sys

sys.path.insert(0, "/opt/trn_rl_repo")

import numpy as np

NCORES = 8
B, O, I = 16, 32, 512
SIZE = 50000
BPC = B // NCORES  # batches per core
V2 = 1250  # lo range (one output tile is 2.5 PSUM banks)
GPT = 4  # groups per output tile: 128 rows = 32 o x 4 groups
SPAN = GPT * V2  # 5000: vocab span per output tile
TILES = SIZE // SPAN  # 10 output tiles per batch
KW = 128  # id-window capacity per (batch, tile)
NW = BPC * TILES  # 20 windows per core
# matmul N-slices of V2, each within one 2 KiB PSUM bank
NSLICES = [(0, 512), (512, 1024), (1024, 1250)]
NWARM = 16  # tensor-engine warmup matmuls (DVFS clock ramp)
LA = 4  # one-hot build lookahead (tiles)
HSPLIT = 800  # copy column split: scalar [0:HSPLIT], vector [HSPLIT:V2]

_cache = {}


def _build(nwarm=NWARM):
    import concourse.bacc as bacc
    import concourse.mybir as mybir
    import concourse.tile as tile

    f32 = mybir.dt.float32
    f16 = mybir.dt.float16
    Alu = mybir.AluOpType

    nc = bacc.Bacc("TRN2", target_bir_lowering=False, debug=False, num_devices=NCORES)

    # host-packed stationary matrices: [b, i_slot, t*KW + (o*4+gl)]
    gt_d = nc.dram_tensor("gtj", [BPC, 128, TILES * KW], f16, kind="ExternalInput").ap()
    # lo = span-relative id mod 1250 per window slot: [p, b*TILES+t] (0 pad)
    lof_d = nc.dram_tensor("lof", [128, NW], f32, kind="ExternalInput").ap()
    out_d = nc.dram_tensor("out", [BPC, O, SIZE], f32, kind="ExternalOutput").ap()

    with tile.TileContext(nc) as tc:
        with (
            tc.tile_pool(name="const", bufs=1) as constp,
            tc.tile_pool(name="idx", bufs=1) as idxp,
            tc.tile_pool(name="alo", bufs=LA + 2) as alop,
            tc.tile_pool(name="outs", bufs=8) as outp,
            tc.tile_pool(name="psmm", bufs=2, space="PSUM") as psmm,
        ):
            # lo-compare constant built on-device, off the input critical
            # path: lov[p, l] = l
            lov = constp.tile([128, V2], f16, tag="lov")
            nc.gpsimd.iota(lov[:], [[1, V2]], channel_multiplier=0,
                           allow_small_or_imprecise_dtypes=True)

            if nwarm:
                warm = constp.tile([128, 256], f16, tag="warm")
                nc.vector.memset(warm[:], 0)
                wps = psmm.tile([128, 256], f32, tag="wm", bufs=1)
                for _ in range(nwarm):
                    nc.tensor.matmul(out=wps[:, :256], lhsT=warm[:, :128],
                                     rhs=warm[:, :256], start=True, stop=True)

            lo_f = idxp.tile([128, NW], f32, tag="lo_f")
            nc.scalar.dma_start(out=lo_f[:], in_=lof_d[:])
            gts = []
            for b in range(BPC):
                t_ = constp.tile([128, TILES * KW], f16, tag=f"gt{b}", name=f"gt{b}")
                nc.sync.dma_start(out=t_[:], in_=gt_d[b])
                gts.append(t_)

            # alo one-hot builds run LA tiles ahead of the matmuls,
            # interleaved with the vector engine's share of the copies
            alos = []

            def build(w):
                alo = alop.tile([128, V2], f16, tag="alo", name=f"alo{w}")
                nc.vector.tensor_scalar(out=alo[:], in0=lov[:],
                                        scalar1=lo_f[:, w : w + 1],
                                        scalar2=None, op0=Alu.is_equal)
                alos.append(alo)

            for w in range(LA):
                build(w)

            for w in range(NW):
                if w + LA < NW:
                    build(w + LA)
                b, t = divmod(w, TILES)
                alo = alos[w]
                gt = gts[b][:, t * KW : (t + 1) * KW]
                ps = psmm.tile([128, V2], f32, tag="mm")
                for n0, n1 in NSLICES:
                    nc.tensor.matmul(out=ps[:, n0:n1], lhsT=gt,
                                     rhs=alo[:, n0:n1], start=True, stop=True)
                os_ = outp.tile([128, V2], f32, tag="os")
                # [32, 4, 1250] view; iteration order (o, g, l) matches
                # the SBUF tile's (partition=(o,g), l) order, and the
                # outer dim of 32 spreads over all 16 SDMA engines
                outv = out_d[b][:, t * SPAN : (t + 1) * SPAN].rearrange(
                    "o (g l) -> o g l", l=V2
                )
                # column-split copy: scalar and vector each move part of
                # every tile (vector also builds, so it gets the smaller
                # share), and each kicks its own half on its own HWDGE queue
                h = HSPLIT
                nc.scalar.copy(out=os_[:, :h], in_=ps[:, :h])
                nc.vector.tensor_copy(out=os_[:, h:], in_=ps[:, h:])
                nc.scalar.dma_start(out=outv[:, :, :h], in_=os_[:, :h])
                nc.sync.dma_start(out=outv[:, :, h:], in_=os_[:, h:])

    nc.compile()
    return nc


def _in_maps(ids, attn):
    lo_w = np.zeros((B, TILES, KW), dtype=np.float32)
    gt_w = np.zeros((B, TILES, KW, KW), dtype=np.float16)  # [.., i_slot, o*4+gl]
    oi = np.arange(O)
    for b in range(B):
        t_of = ids[b] // SPAN
        for t in range(TILES):
            sel = np.nonzero(t_of == t)[0]
            c = sel.size
            if c > KW:
                raise RuntimeError(
                    f"id window overflow: batch {b} span {t} has {c} > {KW} ids"
                )
            rel = ids[b, sel] - t * SPAN
            hi = rel // V2
            lo_w[b, t, :c] = rel % V2
            # gt[i, o*4+gl] = attn[b, o, sel[i]] * (hi[i] == gl)
            cols = attn[b][:, sel].T.astype(np.float16)  # [c, O]
            gt_w[b, t, np.arange(c)[:, None], oi[None, :] * GPT + hi[:, None]] = cols
    lo_t = lo_w.reshape(NCORES, NW, KW).transpose(0, 2, 1)  # [8, 128, NW]
    gt_t = gt_w.reshape(NCORES, BPC, TILES, KW, KW).transpose(
        0, 1, 3, 2, 4
    ).reshape(NCORES, BPC, KW, TILES * KW)
    in_maps = [
        {
            "gtj": np.ascontiguousarray(gt_t[c]),
            "lof": np.ascontiguousarray(lo_t[c]),
        }
        for c in range(NCORES)
    ]
    return in_maps


def kernel(ids, attn):
    from concourse.bass_utils import run_bass_kernel_spmd

    ids = np.ascontiguousarray(ids, dtype=np.int32)
    attn = np.ascontiguousarray(attn, dtype=np.float32)

    if "nc" not in _cache:
        _cache["nc"] = _build()
    nc = _cache["nc"]

    core_ids = list(range(NCORES))
    res = run_bass_kernel_spmd(nc, _in_maps(ids, attn), core_ids)
    out = np.concatenate([res.results[c]["out"] for c in core_ids], axis=0)
    return out


# revision 12
# speedup vs baseline: 1.1040x; 1.1040x over previous
"""Trainium2 Bass kernel for batched weighted scatter-add (AttentionCopy).

Computes out[b, o, v] = sum_i attn[b, o, i] * (ids[b, i] == v)
for ids [16, 512] int32 in [0, 50000), attn [16, 32, 512] f32,
out [16, 32, 50000] f32.

Strategy: pure data parallel over the batch dim — 2 batches per core on 8
cores. Per batch the [32, 50000] output is built densely in 13 PSUM tiles:
12 of [128, 1024] (vocab span 4096 = 4 groups x 1024, exactly 2 PSUM banks
-> pool depth 4, which decouples the matmuls from the copies; a 3-bank
1250-wide variant capped the pool at 2 and stalled the PE every other tile)
plus one [128, 212] tail tile (span 848). Tile rows are o-major (o, gl)
pairs, so each tile's DRAM write is a [32, 4, V2] access pattern whose
outer dim (32) spreads across all 16 SDMA engines (outer-dim count < 16
leaves engines idle — measured 4x DMA slowdown with a g-major pattern).

The host buckets each batch's 512 ids into the 13 spans (index-only
preprocessing; uniform ids put ~42 of 512 in each span, capacity 128) and
packs, per span, the stationary matrix

  gt[i, (o, gl)] = attn[b, o, orig_i] if hi_i == gl else 0     (fp16)

(a masked gather of attn columns — placement only, done on host) plus the
lo = span-relative id mod V2 values. The device then does one K=128 matmul
pass per tile against an on-device one-hot:

  out[(o, gl), lo] = gt.T @ alo,     alo[i, lo] = (lo_i == lo)

This cuts tensor-engine time ~4x vs the K=512 dense formulation (the
original bottleneck: 53us busy of 65us total) and leaves the kernel bounded
by the mandatory 12.8 MB/core f32 output write (~33us at the measured
~390 GB/s aggregate of the 16 SDMA engines, 22.5 GB/s each).

The remaining engine work is balanced so neither scalar nor vector exceeds
the DMA cadence: alo one-hots (vector, LA tiles ahead of the matmuls), and
the PSUM->SBUF copy of each tile column-split scalar [0:640] (slower ACT
copy, no other work) / vector [640:1024], each half DMA-kicked on its own
HWDGE queue. The lo-compare iota constant is generated by gpsimd at t=0.
"""

import sys

sys.path.insert(0, "/opt/trn_rl_repo")

import numpy as np

NCORES = 8
B, O, I = 16, 32, 512
SIZE = 50000
BPC = B // NCORES  # batches per core
V2 = 1024  # full-tile lo range (one output tile = exactly 2 PSUM banks)
V2T = 212  # tail-tile lo range: 12*4*1024 + 4*212 = 50000
GPT = 4  # groups per output tile: 128 rows = 32 o x 4 groups
SPAN = GPT * V2  # 4096: vocab span per full output tile
TILES = 13  # 12 full + 1 tail tile per batch
KW = 128  # id-window capacity per (batch, tile)
NW = BPC * TILES  # 26 windows per core
NWARM = 16  # tensor-engine warmup matmuls (DVFS clock ramp)
LA = 4  # one-hot build lookahead (tiles)
HSPLIT = 640  # copy column split: scalar [0:HSPLIT], vector [HSPLIT:V2]

_cache = {}


def _tile_v2(t):
    return V2 if t < TILES - 1 else V2T


def _tile_off(t):
    return t * SPAN  # tail starts at 12*4096 = 49152


_cache = {}


def _build(nwarm=NWARM):
    import concourse.bacc as bacc
    import concourse.mybir as mybir
    import concourse.tile as tile

    f32 = mybir.dt.float32
    f16 = mybir.dt.float16
    Alu = mybir.AluOpType

    nc = bacc.Bacc("TRN2", target_bir_lowering=False, debug=False, num_devices=NCORES)

    # host-packed stationary matrices: [b, i_slot, t*KW + (o*4+gl)]
    gt_d = nc.dram_tensor("gtj", [BPC, 128, TILES * KW], f16, kind="ExternalInput").ap()
    # lo = span-relative id mod V2 per window slot: [p, b*TILES+t] (0 pad)
    lof_d = nc.dram_tensor("lof", [128, NW], f32, kind="ExternalInput").ap()
    out_d = nc.dram_tensor("out", [BPC, O, SIZE], f32, kind="ExternalOutput").ap()

    with tile.TileContext(nc) as tc:
        with (
            tc.tile_pool(name="const", bufs=1) as constp,
            tc.tile_pool(name="idx", bufs=1) as idxp,
            tc.tile_pool(name="alo", bufs=LA + 2) as alop,
            tc.tile_pool(name="outs", bufs=8) as outp,
            tc.tile_pool(name="psmm", bufs=4, space="PSUM") as psmm,
        ):
            # lo-compare constant built on-device, off the input critical
            # path: lov[p, l] = l
            lov = constp.tile([128, V2], f16, tag="lov")
            nc.gpsimd.iota(lov[:], [[1, V2]], channel_multiplier=0,
                           allow_small_or_imprecise_dtypes=True)

            if nwarm:
                warm = constp.tile([128, 256], f16, tag="warm")
                nc.vector.memset(warm[:], 0)
                wps = psmm.tile([128, V2], f32, tag="mm", name="wps")
                for _ in range(nwarm):
                    nc.tensor.matmul(out=wps[:, :256], lhsT=warm[:, :128],
                                     rhs=warm[:, :256], start=True, stop=True)

            lo_f = idxp.tile([128, NW], f32, tag="lo_f")
            nc.scalar.dma_start(out=lo_f[:], in_=lof_d[:])
            gts = []
            for b in range(BPC):
                t_ = constp.tile([128, TILES * KW], f16, tag=f"gt{b}", name=f"gt{b}")
                nc.sync.dma_start(out=t_[:], in_=gt_d[b])
                gts.append(t_)

            # alo one-hot builds run LA tiles ahead of the matmuls,
            # interleaved with the vector engine's share of the copies
            alos = []

            def build(w):
                v2 = _tile_v2(w % TILES)
                alo = alop.tile([128, V2], f16, tag="alo", name=f"alo{w}")
                nc.vector.tensor_scalar(out=alo[:, :v2], in0=lov[:, :v2],
                                        scalar1=lo_f[:, w : w + 1],
                                        scalar2=None, op0=Alu.is_equal)
                alos.append(alo)

            for w in range(LA):
                build(w)

            for w in range(NW):
                if w + LA < NW:
                    build(w + LA)
                b, t = divmod(w, TILES)
                v2 = _tile_v2(t)
                alo = alos[w]
                gt = gts[b][:, t * KW : (t + 1) * KW]
                ps = psmm.tile([128, V2], f32, tag="mm")
                for n0 in range(0, v2, 512):
                    n1 = min(n0 + 512, v2)
                    nc.tensor.matmul(out=ps[:, n0:n1], lhsT=gt,
                                     rhs=alo[:, n0:n1], start=True, stop=True)
                os_ = outp.tile([128, V2], f32, tag="os")
                # [32, 4, v2] view; iteration order (o, g, l) matches the
                # SBUF tile's (partition=(o,g), l) order, and the outer dim
                # of 32 spreads over all 16 SDMA engines
                off = _tile_off(t)
                outv = out_d[b][:, off : off + GPT * v2].rearrange(
                    "o (g l) -> o g l", l=v2
                )
                if t == TILES - 1:
                    # small tail tile: single copy + kick on the scalar side
                    nc.scalar.copy(out=os_[:, :v2], in_=ps[:, :v2])
                    nc.scalar.dma_start(out=outv, in_=os_[:, :v2])
                    continue
                # column-split copy: scalar and vector each move part of
                # every tile (vector also builds, so it gets the smaller
                # share), and each kicks its own half on its own HWDGE queue
                h = HSPLIT
                nc.scalar.copy(out=os_[:, :h], in_=ps[:, :h])
                nc.vector.tensor_copy(out=os_[:, h:v2], in_=ps[:, h:v2])
                nc.scalar.dma_start(out=outv[:, :, :h], in_=os_[:, :h])
                nc.sync.dma_start(out=outv[:, :, h:], in_=os_[:, h:v2])

    nc.compile()
    return nc


def _in_maps(ids, attn):
    lo_w = np.zeros((B, TILES, KW), dtype=np.float32)
    gt_w = np.zeros((B, TILES, KW, KW), dtype=np.float16)  # [.., i_slot, o*4+gl]
    oi = np.arange(O)
    for b in range(B):
        for t in range(TILES):
            off, v2 = _tile_off(t), _tile_v2(t)
            sel = np.nonzero((ids[b] >= off) & (ids[b] < off + GPT * v2))[0]
            c = sel.size
            if c > KW:
                raise RuntimeError(
                    f"id window overflow: batch {b} span {t} has {c} > {KW} ids"
                )
            rel = ids[b, sel] - off
            hi = rel // v2
            lo_w[b, t, :c] = rel % v2
            # gt[i, o*4+gl] = attn[b, o, sel[i]] * (hi[i] == gl)
            cols = attn[b][:, sel].T.astype(np.float16)  # [c, O]
            gt_w[b, t, np.arange(c)[:, None], oi[None, :] * GPT + hi[:, None]] = cols
    lo_t = lo_w.reshape(NCORES, NW, KW).transpose(0, 2, 1)  # [8, 128, NW]
    gt_t = gt_w.reshape(NCORES, BPC, TILES, KW, KW).transpose(
        0, 1, 3, 2, 4
    ).reshape(NCORES, BPC, KW, TILES * KW)
    in_maps = [
        {
            "gtj": np.ascontiguousarray(gt_t[c]),
            "lof": np.ascontiguousarray(lo_t[c]),
        }
        for c in range(NCORES)
    ]
    return in_maps


def kernel(ids, attn):
    from concourse.bass_utils import run_bass_kernel_spmd

    ids = np.ascontiguousarray(ids, dtype=np.int32)
    attn = np.ascontiguousarray(attn, dtype=np.float32)

    if "nc" not in _cache:
        _cache["nc"] = _build()
    nc = _cache["nc"]

    core_ids = list(range(NCORES))
    res = run_bass_kernel_spmd(nc, _in_maps(ids, attn), core_ids)
    out = np.concatenate([res.results[c]["out"] for c in core_ids], axis=0)
    return out
